# revision 1
# baseline (speedup 1.0000x reference)
"""Causal self-attention (B=2, T=2048, dim=2048, 16 heads, RoPE) on 8 trn2
NeuronCores.

Sharding: core c handles batch b = c//4 and head group g = c%4 (4 heads each,
tensor-parallel over heads). Each core computes QKV projection + RoPE +
causal attention + its partial out-projection; the host sums the 4 partial
out-proj results per batch (the "all-reduce") and stacks batches.

Device kernel layout notes:
  - x is passed transposed per batch: xT [dim, T]. QKV projections produce
    Q^T/K^T [head_dim, T] directly (lhsT = W chunk, rhs = xT chunk).
  - RoPE rotate-half is a signed permutation matmul in [d, t] layout.
  - Attention runs in S^T layout: S^T[k, q] = (K^T chunk)^T-matmul; softmax
    sums are computed with a ones-vector matmul; normalization is applied to
    O^T via a PE broadcast of 1/sums.
  - All big matmuls use float32r (full-rate fp32 on the PE at N>=256).
  - QKV results round-trip through DRAM scratch between the projection and
    attention phases (SBUF cannot hold xT + weights + QKV at once).
"""

import math
import os
import sys
import types

import numpy as np

# ---------------------------------------------------------------------------
# NTFF profile hook (missing antenv.axon_hooks in this image). Reconstructed
# so run_bass_kernel_spmd(trace=True) can measure HW exec time.
# ---------------------------------------------------------------------------
try:
    import antenv

    if "antenv.axon_hooks" not in sys.modules:
        try:
            from trn_agent_boot.trn_boot import _ntff_profile_via_ctypes

            _hook = _ntff_profile_via_ctypes("/opt/axon/libaxon_pjrt.so")
        except Exception:
            _hook = None
        _m = types.ModuleType("antenv.axon_hooks")
        _m.get_axon_ntff_profile_hook = lambda: _hook
        _m.set_axon_ntff_profile_hook = lambda h: None
        sys.modules["antenv.axon_hooks"] = _m
        antenv.axon_hooks = _m
except Exception:
    pass

import concourse.bass as bass
import concourse.tile as tile
from concourse import bacc, mybir
from concourse.bass_utils import run_bass_kernel_spmd
from concourse.masks import make_identity

# Problem constants (hardcoded per the task contract).
B = 2
T = 2048
DIM = 2048
H = 16
HD = 128                  # head_dim
G = 4                     # head groups (cores per batch)
HPG = H // G              # heads per group = 4
N_CORES = 8
SCALE = 1.0 / math.sqrt(HD)

F32 = mybir.dt.float32
F32R = mybir.dt.float32r

TSL = 512                 # t-slice width in the projection phase
NTSL = T // TSL           # 4
QSB = 512                 # query super-block width in the attention phase
NSB = T // QSB            # 4
KC = 128                  # key chunk (partition dim)

LAST_EXEC_NS = None
LAST_RESULTS = None

_PROGRAM_CACHE = {}


def _r(ap):
    return ap.bitcast(F32R)


def _build_program():
    nc = bacc.Bacc("TRN2", target_bir_lowering=False, debug=False,
                   num_devices=N_CORES)

    xT = nc.dram_tensor("xT", [DIM, T], F32R, kind="ExternalInput").ap()
    w_qkv = nc.dram_tensor("w_qkv_loc", [DIM, 3 * HPG * HD], F32R,
                           kind="ExternalInput").ap()
    b_cols = nc.dram_tensor("b_cols", [HD, 3 * HPG], F32,
                            kind="ExternalInput").ap()
    w_out = nc.dram_tensor("w_out_loc", [HPG * HD, DIM], F32R,
                           kind="ExternalInput").ap()
    b_row = nc.dram_tensor("b_out_row", [1, DIM], F32,
                           kind="ExternalInput").ap()
    cosT = nc.dram_tensor("cosT", [HD, T], F32, kind="ExternalInput").ap()
    sinT = nc.dram_tensor("sinT", [HD, T], F32, kind="ExternalInput").ap()
    permT = nc.dram_tensor("permT", [HD, HD], F32R, kind="ExternalInput").ap()
    masks = nc.dram_tensor("masks_t", [KC, QSB // KC, QSB], F32,
                           kind="ExternalInput").ap()
    y = nc.dram_tensor("y_part", [T, DIM], F32, kind="ExternalOutput").ap()

    with tile.TileContext(nc) as tc:
        _emit(tc, nc, xT, w_qkv, b_cols, w_out, b_row, cosT, sinT, permT,
              masks, y)

    nc.compile()
    return nc


def _emit(tc, nc, xT, w_qkv, b_cols_d, w_out, b_row_d, cosT_d, sinT_d,
          permT_d, masks_d, y):
    from contextlib import ExitStack

    ctx = ExitStack()
    with ctx:
        ctx.enter_context(nc.allow_low_precision(
            reason="float32r matmul operands (full-rate fp32 PE)"))
        # ---------------- constants (live for the whole kernel) -----------
        consts = ctx.enter_context(tc.tile_pool(name="consts", bufs=1))
        permT = consts.tile([HD, HD], F32R, tag="permT")
        nc.sync.dma_start(out=permT, in_=permT_d)
        bcols = consts.tile([HD, 3 * HPG], F32, tag="bcols")
        nc.sync.dma_start(out=bcols, in_=b_cols_d)
        ident = consts.tile([KC, KC], F32, tag="ident")
        make_identity(nc, ident)
        ones_f32 = consts.tile([KC, 1], F32, tag="ones_f32")
        nc.vector.memset(ones_f32, 1.0)
        ones_col = consts.tile([KC, 1], F32R, tag="ones_col")
        nc.vector.tensor_copy(ones_col, ones_f32)

        # DRAM scratch for the projection -> attention round-trip.
        dram = ctx.enter_context(tc.tile_pool(name="scr", bufs=1, space="DRAM"))
        # qk_scr[tsl][kind(q=0,k=1), h, d, t_local]
        qk_scr = [dram.tile([2, HPG, HD, TSL], F32R, tag=f"qk{i}", name=f"qk{i}")
                  for i in range(NTSL)]
        # v_scr[tsl][h, j, t_local(128), d]  (V in natural [t, d] layout)
        v_scr = [dram.tile([HPG, TSL // KC, KC, HD], F32R, tag=f"v{i}", name=f"v{i}")
                 for i in range(NTSL)]

        xT_r = xT.rearrange("(c p) t -> p c t", p=KC)        # [128, 16, T]
        w_r = w_qkv.rearrange("(c p) f -> p c f", p=KC)      # [128, 16, 1536]
        NKCH = DIM // KC                                     # 16 k-chunks

        # ======================= Phase A: QKV + RoPE ======================
        with (
            tc.tile_pool(name="a_rope", bufs=1) as a_rope,
            tc.tile_pool(name="a_w", bufs=1) as a_w,
            tc.tile_pool(name="a_x", bufs=2) as a_x,
            tc.tile_pool(name="a_sb", bufs=2) as a_sb,
            tc.tile_pool(name="a_ps", bufs=6, space="PSUM") as a_ps,
            tc.tile_pool(name="a_ps2", bufs=1, space="PSUM") as a_ps2,
            tc.tile_pool(name="a_pst", bufs=1, space="PSUM") as a_pst,
        ):
            # DMA emission order matters for the cold start: x slice 0 and the
            # weight chunks first (they gate the first matmuls), RoPE tables
            # after.
            xsl0 = a_x.tile([KC, NKCH, TSL], F32R, tag="xsl", name="xsl0")
            for jj in range(4):
                nc.sync.dma_start(
                    out=xsl0[:, jj * 4:(jj + 1) * 4, :],
                    in_=xT_r[:, jj * 4:(jj + 1) * 4, 0:TSL])
            # w_qkv_loc is host-packed head-pair-major:
            # [hp0: q(2 heads), k, v | hp1: q, k, v], 256 cols per block.
            # Loading all of hp0's columns first lets the first head-pair's
            # accumulations finish after 6MB of weights instead of 12MB.
            w_all = a_w.tile([KC, NKCH, 3 * HPG * HD], F32R, tag="w_all")
            for hp in range(HPG // 2):
                c0 = hp * 3 * 256
                for kc in range(NKCH):
                    nc.sync.dma_start(out=w_all[:, kc, c0:c0 + 768],
                                      in_=w_r[:, kc, c0:c0 + 768])
            cosT = a_rope.tile([HD, T], F32, tag="cosT")
            nc.sync.dma_start(out=cosT, in_=cosT_d)
            sinT = a_rope.tile([HD, T], F32, tag="sinT")
            nc.sync.dma_start(out=sinT, in_=sinT_d)

            for tsl in range(NTSL):
                t0 = tsl * TSL
                if tsl == 0:
                    xsl = xsl0
                else:
                    xsl = a_x.tile([KC, NKCH, TSL], F32R, tag="xsl")
                    for jj in range(4):
                        nc.sync.dma_start(
                            out=xsl[:, jj * 4:(jj + 1) * 4, :],
                            in_=xT_r[:, jj * 4:(jj + 1) * 4, t0:t0 + TSL])

                for hp in range(HPG // 2):
                    heads = (2 * hp, 2 * hp + 1)
                    outs = [(h, kind) for h in heads for kind in range(3)]
                    # kc-outer: six psum accumulators advance together, so the
                    # PE tracks weight-chunk DMA arrival instead of stalling on
                    # the full 12MB weight load.
                    pstiles = {}
                    for (h, kind) in outs:
                        pstiles[(h, kind)] = a_ps.tile(
                            [HD, TSL], F32, tag="ps_qkv",
                            name=f"ps_{tsl}_{h}_{kind}")
                    for kc in range(NKCH):
                        for (h, kind) in outs:
                            feat0 = (h // 2) * 768 + kind * 256 + (h % 2) * HD
                            nc.tensor.matmul(
                                pstiles[(h, kind)],
                                w_all[:, kc, feat0:feat0 + HD],
                                xsl[:, kc, :],
                                start=(kc == 0), stop=(kc == NKCH - 1),
                            )
                    for h in heads:
                        # ---- Q^T and K^T slices with RoPE ----
                        for kind in (0, 1):
                            ps = pstiles[(h, kind)]
                            qb = a_sb.tile([HD, TSL], F32R, tag="qb")
                            nc.vector.tensor_scalar_add(
                                qb, ps,
                                bcols[:, kind * HPG + h:kind * HPG + h + 1])
                            psr = a_ps2.tile([HD, TSL], F32, tag="ps_rot")
                            nc.tensor.matmul(psr, permT, qb,
                                             start=True, stop=True)
                            m1 = a_sb.tile([HD, TSL], F32, tag="m1")
                            nc.vector.tensor_mul(m1, qb, cosT[:, t0:t0 + TSL])
                            m2 = a_sb.tile([HD, TSL], F32, tag="m2")
                            nc.vector.tensor_mul(m2, psr, sinT[:, t0:t0 + TSL])
                            qo = a_sb.tile([HD, TSL], F32R, tag="qo")
                            nc.vector.tensor_add(qo, m1, m2)
                            nc.sync.dma_start(out=qk_scr[tsl][kind, h], in_=qo)

                        # ---- V slice, transposed to natural [t, d] ----
                        ps = pstiles[(h, 2)]
                        vb = a_sb.tile([HD, TSL], F32, tag="vb")
                        nc.vector.tensor_scalar_add(
                            vb, ps, bcols[:, 2 * HPG + h:2 * HPG + h + 1])
                        for j in range(TSL // KC):
                            pst = a_pst.tile([KC, HD], F32, tag="ps_t")
                            nc.tensor.transpose(
                                pst, vb[:, j * KC:(j + 1) * KC], ident)
                            vn = a_sb.tile([KC, HD], F32R, tag="vn")
                            nc.vector.tensor_copy(vn, pst)
                            nc.sync.dma_start(out=v_scr[tsl][h, j], in_=vn)

        # OT[h]: attention output, transposed [d, t]; survives into phase C.
        ot_pool = ctx.enter_context(tc.tile_pool(name="ot", bufs=1))
        ot = [ot_pool.tile([HD, T], F32R, tag=f"ot{h}", name=f"ot{h}")
              for h in range(HPG)]

        # Phase C weights/bias: loaded early so they arrive during phase B.
        c_w = ctx.enter_context(tc.tile_pool(name="c_w", bufs=1))
        wo = c_w.tile([KC, HPG, DIM], F32R, tag="wo")
        w_out_r = w_out.rearrange("(c p) o -> p c o", p=KC)
        for hc in range(HPG):
            nc.sync.dma_start(out=wo[:, hc, :], in_=w_out_r[:, hc, :])
        brow = c_w.tile([1, DIM], F32, tag="brow")
        nc.sync.dma_start(out=brow, in_=b_row_d)
        bias_bc = c_w.tile([KC, DIM], F32, tag="bias_bc")
        nc.gpsimd.partition_broadcast(bias_bc, brow, channels=KC)

        # ========================= Phase B: attention =====================
        with (
            tc.tile_pool(name="b_consts", bufs=1) as b_consts,
            tc.tile_pool(name="b_qk", bufs=2) as b_qk,
            tc.tile_pool(name="b_pt", bufs=4) as b_pt,
            tc.tile_pool(name="b_sm", bufs=2) as b_sm,
            tc.tile_pool(name="b_ps_s", bufs=2, space="PSUM") as b_ps_s,
            tc.tile_pool(name="b_ps_o", bufs=2, space="PSUM") as b_ps_o,
            tc.tile_pool(name="b_ps_sum", bufs=2, space="PSUM") as b_ps_sum,
        ):
            masks = b_consts.tile([KC, QSB // KC, QSB], F32, tag="masks")
            nc.sync.dma_start(out=masks, in_=masks_d)
            for h in range(HPG):
                qtr = b_qk.tile([HD, T], F32R, tag="qtr")
                ktr = b_qk.tile([HD, T], F32R, tag="ktr")
                vh = b_qk.tile([KC, NKCH, HD], F32R, tag="vh")
                for tsl in range(NTSL):
                    t0 = tsl * TSL
                    nc.sync.dma_start(out=qtr[:, t0:t0 + TSL],
                                      in_=qk_scr[tsl][0, h])
                    nc.sync.dma_start(out=ktr[:, t0:t0 + TSL],
                                      in_=qk_scr[tsl][1, h])
                    for j in range(TSL // KC):
                        nc.sync.dma_start(
                            out=vh[:, tsl * (TSL // KC) + j, :],
                            in_=v_scr[tsl][h, j])

                for sb in range(NSB):
                    q0 = sb * QSB
                    nk = (sb + 1) * (QSB // KC)       # causal key chunks
                    ps_o = b_ps_o.tile([HD, QSB], F32, tag="ps_o")
                    ps_sum = b_ps_sum.tile([1, QSB], F32, tag="ps_sum")
                    for gpair in range(nk // 2):
                        k0 = 2 * gpair
                        ps_st = b_ps_s.tile([KC, 2, QSB], F32, tag="ps_st")
                        for j in range(2):
                            nc.tensor.matmul(
                                ps_st[:, j, :],
                                ktr[:, (k0 + j) * KC:(k0 + j + 1) * KC],
                                qtr[:, q0:q0 + QSB],
                                start=True, stop=True,
                            )
                        pt = b_pt.tile([KC, 2, QSB], F32R, tag="pt")
                        nc.scalar.activation(
                            pt, ps_st, mybir.ActivationFunctionType.Exp,
                            scale=SCALE)
                        for j in range(2):
                            kci = k0 + j
                            dj = kci - (nk - QSB // KC)   # diagonal index
                            if dj >= 0:
                                nc.vector.tensor_mul(
                                    pt[:, j, :], pt[:, j, :], masks[:, dj, :])
                        for j in range(2):
                            kci = k0 + j
                            nc.tensor.matmul(
                                ps_o, vh[:, kci, :], pt[:, j, :],
                                start=(kci == 0), stop=(kci == nk - 1),
                            )
                            nc.tensor.matmul(
                                ps_sum, ones_col, pt[:, j, :],
                                start=(kci == 0), stop=(kci == nk - 1),
                            )
                    # normalize: OT[:, sb] = ps_o * broadcast(1 / sums).
                    # Copy sums out of PSUM, broadcast across partitions, and
                    # only then take the reciprocal -- a [1, N] reciprocal runs
                    # on a single DVE lane (~3.3us), a [128, N] one on all 128.
                    srow = b_sm.tile([1, QSB], F32, tag="srow")
                    nc.scalar.activation(srow, ps_sum,
                                         mybir.ActivationFunctionType.Copy)
                    sbc = b_sm.tile([KC, QSB], F32, tag="sbc")
                    nc.gpsimd.partition_broadcast(sbc, srow, channels=KC)
                    rb = b_sm.tile([KC, QSB], F32, tag="rb")
                    nc.vector.reciprocal(rb, sbc)
                    nc.vector.tensor_mul(ot[h][:, q0:q0 + QSB], ps_o, rb)

        # ========================= Phase C: out-proj ======================
        with (
            tc.tile_pool(name="c_sb", bufs=4) as c_sb,
            tc.tile_pool(name="c_ps", bufs=4, space="PSUM") as c_ps,
        ):
            NOB = DIM // 512
            for tb in range(T // KC):
                tt0 = tb * KC
                for ob in range(NOB):
                    o0 = ob * 512
                    ps_y = c_ps.tile([KC, 512], F32, tag="ps_y")
                    for hc in range(HPG):
                        nc.tensor.matmul(
                            ps_y, ot[hc][:, tt0:tt0 + KC],
                            wo[:, hc, o0:o0 + 512],
                            start=(hc == 0), stop=(hc == HPG - 1),
                        )
                    ys = c_sb.tile([KC, 512], F32, tag="ys")
                    nc.vector.tensor_add(ys, ps_y, bias_bc[:, o0:o0 + 512])
                    nc.sync.dma_start(out=y[tt0:tt0 + KC, o0:o0 + 512],
                                      in_=ys)


# ---------------------------------------------------------------------------
# Host-side input prep
# ---------------------------------------------------------------------------


def _rope_tables():
    inv_freq = 1.0 / (10000.0 ** (np.arange(0, HD, 2, dtype=np.float64) / HD))
    t = np.arange(T, dtype=np.float64)
    freqs = np.outer(t, inv_freq)                     # [T, 64]
    emb = np.concatenate([freqs, freqs], axis=-1)     # [T, 128]
    cosT = np.cos(emb).T.astype(np.float32)           # [128, T]
    sinT = np.sin(emb).T.astype(np.float32)
    return np.ascontiguousarray(cosT), np.ascontiguousarray(sinT)


def _perm_signed_T():
    p = np.zeros((HD, HD), dtype=np.float32)
    half = HD // 2
    for dp in range(half):
        p[dp, dp + half] = -1.0
    for dp in range(half, HD):
        p[dp, dp - half] = 1.0
    return np.ascontiguousarray(p.T)


def _masks_t():
    # masks[r, j, c] = 1 if c >= j*128 + r  (causal mask for the diagonal
    # 512-wide block, per 128-key chunk j)
    r = np.arange(KC)[:, None, None]
    j = np.arange(QSB // KC)[None, :, None]
    c = np.arange(QSB)[None, None, :]
    return (c >= j * KC + r).astype(np.float32)


def kernel(x, w_qkv, b_qkv, w_out, b_out):
    global LAST_EXEC_NS, LAST_RESULTS

    x = np.asarray(x, dtype=np.float32)
    w_qkv = np.asarray(w_qkv, dtype=np.float32)
    b_qkv = np.asarray(b_qkv, dtype=np.float32)
    w_out = np.asarray(w_out, dtype=np.float32)
    b_out = np.asarray(b_out, dtype=np.float32)

    if "prog" not in _PROGRAM_CACHE:
        _PROGRAM_CACHE["prog"] = _build_program()
    nc = _PROGRAM_CACHE["prog"]

    cosT, sinT = _rope_tables()
    permT = _perm_signed_T()
    masks = _masks_t()

    xTs = [np.ascontiguousarray(x[b].T) for b in range(B)]
    in_maps = []
    for c in range(N_CORES):
        b = c // G
        g = c % G
        f0 = g * HPG * HD
        f1 = (g + 1) * HPG * HD
        w_loc = np.ascontiguousarray(np.concatenate(
            [w_qkv[:, base + f0 + hp * 256: base + f0 + (hp + 1) * 256]
             for hp in range(HPG // 2)
             for base in (0, DIM, 2 * DIM)], axis=1))
        b_loc = np.concatenate(
            [b_qkv[f0:f1], b_qkv[DIM + f0:DIM + f1],
             b_qkv[2 * DIM + f0:2 * DIM + f1]])
        b_cols = np.ascontiguousarray(
            b_loc.reshape(3 * HPG, HD).T).astype(np.float32)
        w_out_loc = np.ascontiguousarray(w_out[f0:f1, :])
        b_row = (b_out if g == 0 else np.zeros_like(b_out)).reshape(1, DIM)
        in_maps.append({
            "xT": xTs[b],
            "w_qkv_loc": w_loc,
            "b_cols": b_cols,
            "w_out_loc": w_out_loc,
            "b_out_row": np.ascontiguousarray(b_row),
            "cosT": cosT,
            "sinT": sinT,
            "permT": permT,
            "masks_t": masks,
        })

    trace = bool(os.environ.get("BASS_KERNEL_TRACE"))
    res = run_bass_kernel_spmd(nc, in_maps, list(range(N_CORES)), trace=trace)
    LAST_EXEC_NS = res.exec_time_ns
    LAST_RESULTS = res

    out = np.empty((B, T, DIM), dtype=np.float32)
    for b in range(B):
        acc = res.results[4 * b]["y_part"].astype(np.float32)
        for g in range(1, G):
            acc = acc + res.results[4 * b + g]["y_part"]
        out[b] = acc
    return out



# revision 5
# speedup vs baseline: 1.2108x; 1.2108x over previous
"""Causal self-attention (B=2, T=2048, dim=2048, 16 heads, RoPE) on 8 trn2
NeuronCores.

Sharding: core c handles batch b = c//4 and head group g = c%4 (4 heads each,
tensor-parallel over heads). Each core computes QKV projection + RoPE +
causal attention + its partial out-projection; the host sums the 4 partial
out-proj results per batch (the "all-reduce") and stacks batches.

v2 design (bf16 overhaul):
  - All matmuls in bf16: same PE streaming rate as float32r, but FWL halves
    the per-matmul LDWEIGHTS cost, DMA bytes halve, and DVE elementwise ops
    run at 2x on 16-bit.
  - Q^T/K^T/V stay SBUF-resident between projection and attention (no DRAM
    round trip).
  - RoPE rotate-half is a partition-shifted SBUF->SBUF DMA copy (the sign is
    folded into the host-built sin table), not a PE matmul.
  - V is transposed [d,t]->[t,d] with the DMA XBAR transpose, not PE.
  - Softmax denominators for all 4 heads of a query super-block accumulate
    into one [4, 512] PSUM tile via per-head one-hot ones columns, so one
    [4,512] reciprocal replaces 16 broadcast [128,512] reciprocals.
  - Phases B (attention) and C (out-proj) are merged, super-block-outer:
    each 512-query block's out-projection runs as soon as its softmax is
    normalized, overlapping y DMA writes with later attention.
  - QKV bias is applied on the Scalar engine during PSUM evacuation;
    the output bias is added on the host after the partial sum.
"""

import math
import os
import sys
import types

import numpy as np
import ml_dtypes

BF16NP = ml_dtypes.bfloat16

# ---------------------------------------------------------------------------
# NTFF profile hook (missing antenv.axon_hooks in this image). Reconstructed
# so run_bass_kernel_spmd(trace=True) can measure HW exec time.
# ---------------------------------------------------------------------------
try:
    import antenv

    if "antenv.axon_hooks" not in sys.modules:
        try:
            from trn_agent_boot.trn_boot import _ntff_profile_via_ctypes

            _hook = _ntff_profile_via_ctypes("/opt/axon/libaxon_pjrt.so")
        except Exception:
            _hook = None
        _m = types.ModuleType("antenv.axon_hooks")
        _m.get_axon_ntff_profile_hook = lambda: _hook
        _m.set_axon_ntff_profile_hook = lambda h: None
        sys.modules["antenv.axon_hooks"] = _m
        antenv.axon_hooks = _m
except Exception:
    pass

import concourse.bass as bass
import concourse.tile as tile
from concourse import bacc, mybir
from concourse.bass_utils import run_bass_kernel_spmd

# Problem constants (hardcoded per the task contract).
B = 2
T = 2048
DIM = 2048
H = 16
HD = 128                  # head_dim
G = 4                     # head groups (cores per batch)
HPG = H // G              # heads per group = 4
N_CORES = 8
SCALE = 1.0 / math.sqrt(HD)

F32 = mybir.dt.float32
BF16 = mybir.dt.bfloat16

TSL = 512                 # t-slice width in the projection phase
NTSL = T // TSL           # 4
QSB = 512                 # query super-block width in the attention phase
NSB = T // QSB            # 4
KC = 128                  # key chunk (partition dim)

LAST_EXEC_NS = None
LAST_RESULTS = None

_PROGRAM_CACHE = {}


def _build_program():
    nc = bacc.Bacc("TRN2", target_bir_lowering=False, debug=False,
                   num_devices=N_CORES)

    xT = nc.dram_tensor("xT", [DIM, T], BF16, kind="ExternalInput").ap()
    w_qkv = nc.dram_tensor("w_qkv_loc", [DIM, 3 * HPG * HD], BF16,
                           kind="ExternalInput").ap()
    b_cols = nc.dram_tensor("b_cols", [HD, 3 * HPG], F32,
                            kind="ExternalInput").ap()
    w_out = nc.dram_tensor("w_out_loc", [HPG * HD, DIM], BF16,
                           kind="ExternalInput").ap()
    cosT = nc.dram_tensor("cosT", [HD, T], F32, kind="ExternalInput").ap()
    sinT = nc.dram_tensor("sinTs", [HD, T], F32, kind="ExternalInput").ap()
    masks = nc.dram_tensor("masks_t", [KC, QSB // KC, QSB], BF16,
                           kind="ExternalInput").ap()
    y = nc.dram_tensor("y_part", [T, DIM], BF16, kind="ExternalOutput").ap()

    with tile.TileContext(nc) as tc:
        _emit(tc, nc, xT, w_qkv, b_cols, w_out, cosT, sinT, masks, y)

    nc.compile()
    return nc


def _emit(tc, nc, xT, w_qkv, b_cols_d, w_out, cosT_d, sinT_d, masks_d, y):
    from contextlib import ExitStack

    ctx = ExitStack()
    with ctx:
        ctx.enter_context(nc.allow_low_precision(
            reason="bf16 matmul operands and elementwise pipeline"))
        # ---------------- constants (live for the whole kernel) -----------
        consts = ctx.enter_context(tc.tile_pool(name="consts", bufs=1))
        bcols = consts.tile([HD, 3 * HPG], F32, tag="bcols")
        nc.sync.dma_start(out=bcols, in_=b_cols_d)
        # ones4[:, h, :] is the [128, 4] one-hot stationary for head h: only
        # column h is ones, so head h's softmax-sum matmul lands in row h of
        # the shared [HPG, QSB] PSUM accumulator (other rows accumulate +0).
        ones4 = consts.tile([KC, HPG, HPG], BF16, tag="ones4")
        nc.vector.memset(ones4, 0.0)
        for h in range(HPG):
            nc.vector.memset(ones4[:, h, h:h + 1], 1.0)
        masks_sb = consts.tile([KC, QSB // KC, QSB], BF16, tag="masks")
        nc.sync.dma_start(out=masks_sb, in_=masks_d)

        # QKV, attention output: SBUF-resident for the whole kernel.
        qkv_pool = ctx.enter_context(tc.tile_pool(name="qkv", bufs=1))
        qtr = [qkv_pool.tile([HD, T], BF16, tag=f"qtr{h}", name=f"qtr{h}")
               for h in range(HPG)]
        ktr = [qkv_pool.tile([HD, T], BF16, tag=f"ktr{h}", name=f"ktr{h}")
               for h in range(HPG)]
        vh = [qkv_pool.tile([KC, T // KC, HD], BF16, tag=f"vh{h}",
                            name=f"vh{h}")
              for h in range(HPG)]

        rope = ctx.enter_context(tc.tile_pool(name="rope", bufs=1))
        cosT = rope.tile([HD, T], F32, tag="cosT")
        sinT = rope.tile([HD, T], F32, tag="sinT")

        xT_r = xT.rearrange("(c p) t -> p c t", p=KC)        # [128, 16, T]
        w_r = w_qkv.rearrange("(c p) f -> p c f", p=KC)      # [128, 16, 1536]
        NKCH = DIM // KC                                     # 16 k-chunks

        # ======================= Phase A: QKV + RoPE ======================
        with (
            tc.tile_pool(name="a_w", bufs=1) as a_w,
            tc.tile_pool(name="a_x", bufs=2) as a_x,
            tc.tile_pool(name="a_sb", bufs=2) as a_sb,
            tc.tile_pool(name="a_ps", bufs=8, space="PSUM") as a_ps,
        ):
            # DMA emission order matters for the cold start: x slice 0 and the
            # weight chunks first (they gate the first matmuls), RoPE tables
            # after.
            xsl0 = a_x.tile([KC, NKCH, TSL], BF16, tag="xsl", name="xsl0")
            for jj in range(4):
                nc.sync.dma_start(
                    out=xsl0[:, jj * 4:(jj + 1) * 4, :],
                    in_=xT_r[:, jj * 4:(jj + 1) * 4, 0:TSL])
            # w_qkv_loc is host-packed head-pair-major:
            # [hp0: q(2 heads), k, v | hp1: q, k, v], 256 cols per block.
            w_all = a_w.tile([KC, NKCH, 3 * HPG * HD], BF16, tag="w_all")
            for hp in range(HPG // 2):
                c0 = hp * 3 * 256
                for kc in range(NKCH):
                    nc.sync.dma_start(out=w_all[:, kc, c0:c0 + 768],
                                      in_=w_r[:, kc, c0:c0 + 768])
            nc.sync.dma_start(out=cosT, in_=cosT_d)
            nc.sync.dma_start(out=sinT, in_=sinT_d)

            for tsl in range(NTSL):
                t0 = tsl * TSL
                if tsl == 0:
                    xsl = xsl0
                else:
                    xsl = a_x.tile([KC, NKCH, TSL], BF16, tag="xsl")
                    for jj in range(4):
                        nc.sync.dma_start(
                            out=xsl[:, jj * 4:(jj + 1) * 4, :],
                            in_=xT_r[:, jj * 4:(jj + 1) * 4, t0:t0 + TSL])

                for hp in range(HPG // 2):
                    heads = (2 * hp, 2 * hp + 1)
                    outs = [(h, kind) for h in heads for kind in range(3)]
                    # kc-outer: six psum accumulators advance together, so the
                    # PE tracks weight-chunk DMA arrival instead of stalling on
                    # the full weight load.
                    pstiles = {}
                    for (h, kind) in outs:
                        pstiles[(h, kind)] = a_ps.tile(
                            [HD, TSL], F32, tag="ps_qkv",
                            name=f"ps_{tsl}_{h}_{kind}")
                    for kc in range(NKCH):
                        for (h, kind) in outs:
                            feat0 = (h // 2) * 768 + kind * 256 + (h % 2) * HD
                            nc.tensor.matmul(
                                pstiles[(h, kind)],
                                w_all[:, kc, feat0:feat0 + HD],
                                xsl[:, kc, :],
                                start=(kc == 0), stop=(kc == NKCH - 1),
                            )
                    for h in heads:
                        # ---- Q^T and K^T slices with RoPE ----
                        for kind in (0, 1):
                            ps = pstiles[(h, kind)]
                            dst = qtr[h] if kind == 0 else ktr[h]
                            # bias add on the Scalar engine (psum evac)
                            qb = a_sb.tile([HD, TSL], F32, tag="qb")
                            nc.scalar.activation(
                                qb, ps, mybir.ActivationFunctionType.Identity,
                                bias=bcols[:, kind * HPG + h:
                                           kind * HPG + h + 1])
                            # rotate-half: partition-shifted SBUF->SBUF copy
                            # (sign folded into the host-built sin table)
                            qrot = a_sb.tile([HD, TSL], F32, tag="qrot")
                            half = HD // 2
                            nc.sync.dma_start(out=qrot[0:half, :],
                                              in_=qb[half:HD, :])
                            nc.sync.dma_start(out=qrot[half:HD, :],
                                              in_=qb[0:half, :])
                            m1 = a_sb.tile([HD, TSL], F32, tag="m1")
                            nc.vector.tensor_mul(m1, qb, cosT[:, t0:t0 + TSL])
                            m2 = a_sb.tile([HD, TSL], F32, tag="m2")
                            nc.vector.tensor_mul(m2, qrot,
                                                 sinT[:, t0:t0 + TSL])
                            nc.vector.tensor_add(dst[:, t0:t0 + TSL], m1, m2)

                        # ---- V slice -> bf16, then XBAR-transpose to [t, d]
                        ps = pstiles[(h, 2)]
                        vb = a_sb.tile([HD, TSL], BF16, tag="vb")
                        nc.scalar.activation(
                            vb, ps, mybir.ActivationFunctionType.Identity,
                            bias=bcols[:, 2 * HPG + h:2 * HPG + h + 1])
                        for j in range(TSL // KC):
                            nc.sync.dma_start_transpose(
                                out=vh[h][:, tsl * (TSL // KC) + j, :],
                                in_=vb[:, j * KC:(j + 1) * KC])

        # ================= Phase B+C: attention + out-proj ================
        # wo loaded at phase-B start; arrives during the first super-block.
        c_w = ctx.enter_context(tc.tile_pool(name="c_w", bufs=1))
        wo = c_w.tile([KC, HPG, DIM], BF16, tag="wo")
        w_out_r = w_out.rearrange("(c p) o -> p c o", p=KC)
        for hc in range(HPG):
            nc.sync.dma_start(out=wo[:, hc, :], in_=w_out_r[:, hc, :])

        with (
            tc.tile_pool(name="b_pt", bufs=3) as b_pt,
            tc.tile_pool(name="b_ot", bufs=2) as b_ot,
            tc.tile_pool(name="b_sm", bufs=2) as b_sm,
            tc.tile_pool(name="c_sb", bufs=4) as c_sb,
            tc.tile_pool(name="b_ps_s", bufs=2, space="PSUM") as b_ps_s,
            tc.tile_pool(name="b_ps_o", bufs=1, space="PSUM") as b_ps_o,
            tc.tile_pool(name="b_ps_sum", bufs=1, space="PSUM") as b_ps_sum,
            tc.tile_pool(name="c_ps", bufs=2, space="PSUM") as c_ps,
        ):
            for sb in range(NSB):
                q0 = sb * QSB
                nk = (sb + 1) * (QSB // KC)       # causal key chunks
                # all 4 heads' softmax sums accumulate into one [4, 512]
                ps_sum = b_ps_sum.tile([HPG, QSB], F32, tag="ps_sum")
                otu = []
                for h in range(HPG):
                    ps_o = b_ps_o.tile([HD, QSB], F32, tag="ps_o")
                    for gpair in range(nk // 2):
                        k0 = 2 * gpair
                        ps_st = b_ps_s.tile([KC, 2, QSB], F32, tag="ps_st")
                        for j in range(2):
                            nc.tensor.matmul(
                                ps_st[:, j, :],
                                ktr[h][:, (k0 + j) * KC:(k0 + j + 1) * KC],
                                qtr[h][:, q0:q0 + QSB],
                                start=True, stop=True,
                            )
                        pt = b_pt.tile([KC, 2, QSB], BF16, tag="pt")
                        nc.scalar.activation(
                            pt, ps_st, mybir.ActivationFunctionType.Exp,
                            scale=SCALE)
                        for j in range(2):
                            kci = k0 + j
                            dj = kci - (nk - QSB // KC)   # diagonal index
                            if dj >= 0:
                                nc.vector.tensor_mul(
                                    pt[:, j, :], pt[:, j, :],
                                    masks_sb[:, dj, :])
                        for j in range(2):
                            kci = k0 + j
                            nc.tensor.matmul(
                                ps_o, vh[h][:, kci, :], pt[:, j, :],
                                start=(kci == 0), stop=(kci == nk - 1),
                            )
                            nc.tensor.matmul(
                                ps_sum, ones4[:, h, :], pt[:, j, :],
                                start=(h == 0 and kci == 0),
                                stop=(h == HPG - 1 and kci == nk - 1),
                            )
                    # evacuate unnormalized O^T (bf16); normalized after the
                    # batched reciprocal below.
                    ou = b_ot.tile([HD, QSB], BF16, tag=f"otu{h}")
                    nc.vector.tensor_copy(ou, ps_o)
                    otu.append(ou)

                # batched reciprocal: one [4, 512] op for all heads
                rsums = b_sm.tile([HPG, QSB], F32, tag="rsums")
                nc.vector.reciprocal(rsums, ps_sum)
                otn = []
                for h in range(HPG):
                    # partition_broadcast requires its input at partition 0:
                    # stage row h there with a tiny DMA first.
                    r1 = b_sm.tile([1, QSB], F32, tag="r1")
                    nc.sync.dma_start(out=r1, in_=rsums[h:h + 1, :])
                    rb = b_sm.tile([KC, QSB], F32, tag="rb")
                    nc.gpsimd.partition_broadcast(rb, r1, channels=KC)
                    on = b_ot.tile([HD, QSB], BF16, tag=f"otn{h}")
                    nc.vector.tensor_mul(on, otu[h], rb)
                    otn.append(on)

                # out-projection for this super-block's 512 queries
                for tb in range(QSB // KC):
                    tt0 = tb * KC
                    for ob in range(DIM // 512):
                        o0 = ob * 512
                        ps_y = c_ps.tile([KC, 512], F32, tag="ps_y")
                        for hc in range(HPG):
                            nc.tensor.matmul(
                                ps_y, otn[hc][:, tt0:tt0 + KC],
                                wo[:, hc, o0:o0 + 512],
                                start=(hc == 0), stop=(hc == HPG - 1),
                            )
                        ys = c_sb.tile([KC, 512], BF16, tag="ys")
                        # alternate evac engine: scalar/vector
                        if ob % 2 == 0:
                            nc.scalar.activation(
                                ys, ps_y,
                                mybir.ActivationFunctionType.Identity)
                        else:
                            nc.vector.tensor_copy(ys, ps_y)
                        nc.sync.dma_start(
                            out=y[q0 + tt0:q0 + tt0 + KC, o0:o0 + 512],
                            in_=ys)


# ---------------------------------------------------------------------------
# Host-side input prep
# ---------------------------------------------------------------------------


def _rope_tables():
    inv_freq = 1.0 / (10000.0 ** (np.arange(0, HD, 2, dtype=np.float64) / HD))
    t = np.arange(T, dtype=np.float64)
    freqs = np.outer(t, inv_freq)                     # [T, 64]
    emb = np.concatenate([freqs, freqs], axis=-1)     # [T, 128]
    cosT = np.cos(emb).T.astype(np.float32)           # [128, T]
    sinT = np.sin(emb).T.astype(np.float32)
    # rotate_half(x) = [-x2, x1]; the device computes qrot = [x2, x1], so
    # fold the sign of the first half into the sin table.
    sinT[:HD // 2, :] *= -1.0
    return np.ascontiguousarray(cosT), np.ascontiguousarray(sinT)


def _masks_t():
    # masks[r, j, c] = 1 if c >= j*128 + r  (causal mask for the diagonal
    # 512-wide block, per 128-key chunk j)
    r = np.arange(KC)[:, None, None]
    j = np.arange(QSB // KC)[None, :, None]
    c = np.arange(QSB)[None, None, :]
    return (c >= j * KC + r).astype(BF16NP)


def kernel(x, w_qkv, b_qkv, w_out, b_out):
    global LAST_EXEC_NS, LAST_RESULTS

    x = np.asarray(x, dtype=np.float32)
    w_qkv = np.asarray(w_qkv, dtype=np.float32)
    b_qkv = np.asarray(b_qkv, dtype=np.float32)
    w_out = np.asarray(w_out, dtype=np.float32)
    b_out = np.asarray(b_out, dtype=np.float32)

    if "prog" not in _PROGRAM_CACHE:
        _PROGRAM_CACHE["prog"] = _build_program()
    nc = _PROGRAM_CACHE["prog"]

    cosT, sinT = _rope_tables()
    masks = _masks_t()

    xTs = [np.ascontiguousarray(x[b].T.astype(BF16NP)) for b in range(B)]
    in_maps = []
    for c in range(N_CORES):
        b = c // G
        g = c % G
        f0 = g * HPG * HD
        f1 = (g + 1) * HPG * HD
        w_loc = np.ascontiguousarray(np.concatenate(
            [w_qkv[:, base + f0 + hp * 256: base + f0 + (hp + 1) * 256]
             for hp in range(HPG // 2)
             for base in (0, DIM, 2 * DIM)], axis=1).astype(BF16NP))
        b_loc = np.concatenate(
            [b_qkv[f0:f1], b_qkv[DIM + f0:DIM + f1],
             b_qkv[2 * DIM + f0:2 * DIM + f1]])
        b_cols = np.ascontiguousarray(
            b_loc.reshape(3 * HPG, HD).T).astype(np.float32)
        w_out_loc = np.ascontiguousarray(w_out[f0:f1, :].astype(BF16NP))
        in_maps.append({
            "xT": xTs[b],
            "w_qkv_loc": w_loc,
            "b_cols": b_cols,
            "w_out_loc": w_out_loc,
            "cosT": cosT,
            "sinTs": sinT,
            "masks_t": masks,
        })

    trace = bool(os.environ.get("BASS_KERNEL_TRACE"))
    res = run_bass_kernel_spmd(nc, in_maps, list(range(N_CORES)), trace=trace)
    LAST_EXEC_NS = res.exec_time_ns
    LAST_RESULTS = res

    out = np.empty((B, T, DIM), dtype=np.float32)
    for b in range(B):
        acc = res.results[4 * b]["y_part"].astype(np.float32)
        for g in range(1, G):
            acc = acc + res.results[4 * b + g]["y_part"].astype(np.float32)
        out[b] = acc + b_out[None, :]
    return out
